# revision 1
# baseline (speedup 1.0000x reference)
"""Trainium2 Bass kernel for the recurrent STP network (nn_Network_20109036880204).

Strategy: tensor-parallel over the output-neuron dim across 8 NeuronCores.
  - Each core owns a 1024-neuron shard: W_c = Wab[c*1024:(c+1)*1024, :]^T,
    stored fp16 resident in SBUF as 64 K-tiles [128, 1024] (128 KiB/partition).
  - All [B, N] state tensors live in SBUF in "state layout": tile [128, 256]
    with  tile[p, j*32 + b] = state[b, n = c*1024 + j*128 + p].
    (n on partitions -> fast 128-lane elementwise AND the matmul's stationary
    operand y^T [128, 32] is a contiguous free-dim slice.)
  - Per step: y = u'*x'*r (fp16) -> DRAM -> AllGather(8) -> y_full in SBUF ->
    128 matmuls (K=8192 in 64 tiles, N=1024 in 2 PSUM chunks) -> PE transpose
    of the [32, 1024] result back into state layout -> fused DVE update chain.
"""

import sys

for _p in ("/opt/trn_rl_repo", "/root/.axon_site/_ro/trn_rl_repo"):
    if _p not in sys.path:
        sys.path.append(_p)

import numpy as np

import concourse.bass as bass
import concourse.bacc as bacc
import concourse.mybir as mybir
import concourse.tile as tile
from concourse import bass_utils, masks

# problem constants
NCORES = 8
B = 32
N = 8192
NS = N // NCORES          # 1024 neurons per core
P = 128
J = NS // P               # 8 local K-tiles per core
T = N // P                # 64 K-tiles total
F = J * B                 # 256 = free size of a state tile
CHUNK = 512               # matmul moving free dim (one PSUM bank)
NCH = NS // CHUNK         # 2 chunks

DT = 0.01
USE = 0.03
TAU_FAC = 1.0
TAU_REC = 0.25
C1 = DT / TAU_FAC         # 0.01
C0 = DT * USE / TAU_FAC   # 3e-4
A1 = USE * DT             # 3e-4
C2 = DT / TAU_REC         # 0.04

F32 = mybir.dt.float32
F16 = mybir.dt.float16
MULT = mybir.AluOpType.mult
ADD = mybir.AluOpType.add
MAX = mybir.AluOpType.max


# Skewed A/B split: half A = first JA j-blocks (gathered early, small so its
# AllGather completes by matmul-end), half B = the rest.
JA = 3
JB = J - JA
HA = JA * B               # 96  = state-free width of half A
HB = JB * B               # 160 = width of half B
HW = {"A": HA, "B": HB}
A_TILES = [t for t in range(T) if t % J < JA]
B_TILES = [t for t in range(T) if t % J >= JA]
# output column groups (psum free widths; each <= 512 = one bank)
G_BOUNDS = [0, JA * P, JA * P + 4 * P, NS]      # [0, 384, 896, 1024]
NG = len(G_BOUNDS) - 1
# j-block -> group index
J_GROUP = [next(g for g in range(NG)
                if G_BOUNDS[g] <= j * P < G_BOUNDS[g + 1]) for j in range(J)]


def build_program(n_steps: int, uni=(None, None, None, None), n_dummy=10):
    """Build the SPMD Bass program (identical on all 8 cores).

    Two-half pipeline: each core's y shard is split into half A (j=0..3)
    and half B (j=4..7); each half is all-gathered separately so AG_A can
    fly while the tail of the matmul still runs, and the next step's
    matmul consumes A-sourced K-tiles first.
    """
    es_v, ds_v, e_v, dt_v = uni  # uniform values of the const vectors, or None

    nc = bacc.Bacc(
        "TRN2",
        target_bir_lowering=False,
        debug=False,
        num_devices=NCORES,
    )

    w_dram = nc.dram_tensor("w", [T, P, NS], F16, kind="ExternalInput")
    sd = {
        nm: nc.dram_tensor(nm, [P, F], F32, kind="ExternalInput")
        for nm in ["r0", "recs0", "u0", "x0", "ff", "es", "ds", "e", "dt"]
    }
    r_out = nc.dram_tensor("r_out", [P, F], F32, kind="ExternalOutput")

    with tile.TileContext(nc) as tc:
        with (
            tc.tile_pool(name="wpool", bufs=1) as wpool,
            tc.tile_pool(name="cpool", bufs=1) as cpool,
            tc.tile_pool(name="spool", bufs=2) as spool,
            tc.tile_pool(name="wk", bufs=2) as wk,
            tc.tile_pool(name="yp", bufs=2) as yp,
            tc.tile_pool(name="pmm", bufs=2, space="PSUM") as pmm,
            tc.tile_pool(name="pT", bufs=2, space="PSUM") as pT,
            tc.tile_pool(name="dp", bufs=3, space="DRAM") as dp,
        ):
            # ---- resident weights: 16 DMAs so they spread across queues ----
            w_sb = wpool.tile([P, T * NS], F16, tag="w")
            TB = 4  # K-tiles per DMA
            for i in range(T // TB):
                dst = w_sb[:, i * TB * NS:(i + 1) * TB * NS].rearrange(
                    "p (t n) -> p t n", t=TB
                )
                src = w_dram[i * TB:(i + 1) * TB, :, :].rearrange("t p n -> p t n")
                nc.sync.dma_start(dst, src)

            # ---- constants / initial state ----
            ff_sb = cpool.tile([P, F], F32, tag="ff")
            es_sb = cpool.tile([P, F], F32, tag="es")
            ds_sb = cpool.tile([P, F], F32, tag="ds")
            e_sb = cpool.tile([P, F], F32, tag="e")
            dt_sb = cpool.tile([P, F], F32, tag="dt")
            ident = cpool.tile([B, B], F32, tag="ident")
            for t_, nm in [(ff_sb, "ff"), (es_sb, "es"), (ds_sb, "ds"),
                           (e_sb, "e"), (dt_sb, "dt")]:
                nc.sync.dma_start(t_[:], sd[nm][:])
            masks.make_identity(nc, ident[:])

            r = spool.tile([P, F], F32, tag="r")
            recS = spool.tile([P, F], F32, tag="recS")
            u0_sb = wk.tile([P, F], F32, tag="u0", bufs=1)
            x0_sb = wk.tile([P, F], F32, tag="x0", bufs=1)
            for t_, nm in [(r, "r0"), (recS, "recs0"), (u0_sb, "u0"),
                           (x0_sb, "x0")]:
                nc.sync.dma_start(t_[:], sd[nm][:])

            V = nc.vector

            # ---- prologue: u1, x1, y0 from initial state ----
            s1 = wk.tile([P, F], F32, tag="t0", bufs=1)
            m = wk.tile([P, F], F32, tag="t1", bufs=1)
            s2 = wk.tile([P, F], F32, tag="t2", bufs=1)
            un = spool.tile([P, F], F32, tag="u")
            V.tensor_scalar(s1[:], u0_sb[:], 1.0 - C1, C0, MULT, ADD)
            V.tensor_mul(m[:], u0_sb[:], r[:])
            V.scalar_tensor_tensor(s2[:], r[:], A1, s1[:], MULT, ADD)
            V.scalar_tensor_tensor(un[:], m[:], -A1, s2[:], MULT, ADD)

            t2p = wk.tile([P, F], F32, tag="t3", bufs=1)
            t3p = wk.tile([P, F], F32, tag="t4", bufs=1)
            s4 = wk.tile([P, F], F32, tag="t5", bufs=1)
            xn = spool.tile([P, F], F32, tag="x")
            V.tensor_mul(t2p[:], x0_sb[:], r[:])
            V.tensor_mul(t3p[:], un[:], t2p[:])
            V.tensor_scalar(s4[:], x0_sb[:], 1.0 - C2, C2, MULT, ADD)
            V.scalar_tensor_tensor(xn[:], t3p[:], -DT, s4[:], MULT, ADD)

            w0 = wk.tile([P, F], F32, tag="t6", bufs=1)
            yh = {}
            V.tensor_mul(w0[:], un[:], xn[:])
            for hf, sl in (("A", slice(0, HA)), ("B", slice(HA, F))):
                yh[hf] = yp.tile([P, HW[hf]], F16, tag=f"y{hf}",
                                 name=f"y{hf}_pro")
                V.tensor_mul(yh[hf][:], w0[:, sl], r[:, sl])

            ag_counter = [0]

            def launch_ag(hf, ytile):
                """store y-half to DRAM, AllGather, DMA gathered tiles back."""
                k = ag_counter[0] = ag_counter[0] + 1
                w_ = HW[hf]
                ydr = dp.tile([P, w_], F16, tag=f"ydr{hf}", name=f"ydr{hf}_{k}")
                nc.sync.dma_start(ydr[:], ytile[:])
                yall = dp.tile([NCORES, P, w_], F16, tag=f"yall{hf}",
                               name=f"yall{hf}_{k}")
                nc.gpsimd.collective_compute(
                    "AllGather",
                    mybir.AluOpType.bypass,
                    replica_groups=[list(range(NCORES))],
                    ins=[ydr.opt()],
                    outs=[yall.opt()],
                )
                yfull = yp.tile([P, NCORES * w_], F16, tag=f"yfull{hf}",
                                name=f"yfull{hf}_{k}")
                # block c=0 first (tiny DMA) so the next step's first
                # matmuls ungate as early as possible
                nc.sync.dma_start(yfull[:, :w_], yall[0, :, :])
                nc.sync.dma_start(
                    yfull[:, w_:].rearrange("p (c f) -> p c f", c=NCORES - 1),
                    yall[1:, :, :].rearrange("c p f -> p c f"),
                )
                return yfull

            yfullA = launch_ag("A", yh["A"])
            yfullB = launch_ag("B", yh["B"])

            pdum = pmm.tile([B, CHUNK], F32, tag="dummy", bufs=1,
                            name="pdum") if n_dummy else None

            def lhst_ap(yfA, yfB, t):
                c, j = divmod(t, J)
                if j < JA:
                    return yfA[:, c * HA + j * B:c * HA + (j + 1) * B]
                jb = j - JA
                return yfB[:, c * HB + jb * B:c * HB + (jb + 1) * B]

            # ---- main loop ----
            for it in range(n_steps):
                last = it == n_steps - 1

                # precompute (overlaps AG + matmul on DVE)
                A_t = wk.tile([P, F], F32, tag="A", bufs=1)
                B_t = wk.tile([P, F], F32, tag="B", bufs=1)
                C_t = wk.tile([P, F], F32, tag="C", bufs=1)
                D_t = wk.tile([P, F], F32, tag="D", bufs=1)
                rE = wk.tile([P, F], F32, tag="rE", bufs=1)
                if not last:
                    V.tensor_scalar(A_t[:], un[:], 1.0 - C1, C0, MULT, ADD)
                    V.tensor_scalar(B_t[:], un[:], -A1, A1, MULT, ADD)
                    V.tensor_scalar(C_t[:], xn[:], 1.0 - C2, C2, MULT, ADD)
                    V.tensor_scalar(D_t[:], xn[:], DT, None, MULT)
                if e_v is None:
                    V.tensor_mul(rE[:], r[:], e_sb[:])

                # dummy matmuls: PE/HAM-warming filler during the AllGather
                # wait at the head of each step (read-only on w_sb)
                if n_dummy and it > 0:
                    for _ in range(n_dummy):
                        nc.tensor.matmul(
                            pdum[:], lhsT=w_sb[:, :B], rhs=w_sb[:, :CHUNK],
                            start=True, stop=True,
                        )

                # matmul: NG output-column groups x 64 K-tiles. Order:
                # [all groups : A-sourced K-tiles] [G0 : B-sourced]
                # -> G0 stops early; its transposes/ew/AllGather fly under
                # the remaining B-sourced matmuls of G1/G2.
                pm = [pmm.tile([B, G_BOUNDS[g + 1] - G_BOUNDS[g]], F32,
                               tag=f"mm{g}", name=f"pm{g}_{it}",
                               bufs=(2 if g < 2 else 1))
                      for g in range(NG)]
                nmm = [0] * NG

                def emit_group(g, tiles):
                    lo, hi = G_BOUNDS[g], G_BOUNDS[g + 1]
                    for t in tiles:
                        nc.tensor.matmul(
                            pm[g][:],
                            lhsT=lhst_ap(yfullA, yfullB, t),
                            rhs=w_sb[:, t * NS + lo:t * NS + hi],
                            start=(nmm[g] == 0),
                            stop=(nmm[g] == T - 1),
                        )
                        nmm[g] += 1

                def transpose_jblocks(hf, jlist):
                    """PSUM group columns -> state-layout PSUM [128, HW[hf]].

                    Per-j 128-col ACT copies so each PE transpose only waits
                    on its own small copy (~0.2us), not a whole chunk.
                    """
                    mmT_ = pT.tile([P, HW[hf]], F32, tag=f"mmT{hf}", bufs=1,
                                   name=f"mmT{hf}_{it}")
                    stage = wk.tile([B, len(jlist) * P], F32, tag=f"stage{hf}",
                                    bufs=1, name=f"stage{hf}_{it}")
                    done_g = set()
                    for k_, j in enumerate(jlist):
                        g = J_GROUP[j]
                        if g not in done_g:
                            done_g.add(g)
                            lo = max(G_BOUNDS[g], jlist[0] * P)
                            hi = min(G_BOUNDS[g + 1], (jlist[-1] + 1) * P)
                            nc.scalar.copy(
                                stage[:, lo - jlist[0] * P:hi - jlist[0] * P],
                                pm[g][:, lo - G_BOUNDS[g]:hi - G_BOUNDS[g]])
                        nc.tensor.transpose(
                            mmT_[:, k_ * B:(k_ + 1) * B],
                            stage[:, k_ * P:(k_ + 1) * P],
                            ident[:],
                        )
                    return mmT_

                for g in range(NG):
                    emit_group(g, A_TILES)
                emit_group(0, B_TILES)
                mmTA = transpose_jblocks("A", list(range(JA)))
                emit_group(1, B_TILES)
                emit_group(2, B_TILES)

                # names for per-half state pieces of this iteration
                rec_new = spool.tile([P, F], F32, tag="recfull")
                r_new = spool.tile([P, F], F32, tag="r")
                recS_new = spool.tile([P, F], F32, tag="recS")
                q = spool.tile([P, F], F32, tag="u")
                v = spool.tile([P, F], F32, tag="x")
                newy = {"A": yp.tile([P, HA], F16, tag="yA", name=f"yA_{it}"),
                        "B": yp.tile([P, HB], F16, tag="yB", name=f"yB_{it}")}

                def ew_half(hf, mmT_half):
                    sl = slice(0, HA) if hf == "A" else slice(HA, F)
                    HF = HW[hf]
                    if ds_v is not None:
                        V.scalar_tensor_tensor(rec_new[:, sl], mmT_half[:],
                                               ds_v, recS[:, sl], MULT, ADD)
                    else:
                        tmp = wk.tile([P, HF], F32, tag=f"w0{hf}", bufs=1)
                        V.tensor_mul(tmp[:], mmT_half[:], ds_sb[:, sl])
                        V.tensor_add(rec_new[:, sl], tmp[:], recS[:, sl])
                    h_ = wk.tile([P, HF], F32, tag=f"w1{hf}", bufs=1)
                    V.tensor_add(h_[:], rec_new[:, sl], ff_sb[:, sl])
                    dr_ = wk.tile([P, HF], F32, tag=f"w2{hf}", bufs=1)
                    if dt_v is not None:
                        V.tensor_scalar(dr_[:], h_[:], 0.0, dt_v, MAX, MULT)
                    else:
                        V.scalar_tensor_tensor(dr_[:], h_[:], 0.0, dt_sb[:, sl],
                                               MAX, MULT)
                    if e_v is not None:
                        V.scalar_tensor_tensor(r_new[:, sl], r[:, sl], e_v,
                                               dr_[:], MULT, ADD)
                    else:
                        V.tensor_add(r_new[:, sl], dr_[:], rE[:, sl])
                    if last:
                        return None
                    if es_v is not None:
                        V.tensor_scalar(recS_new[:, sl], rec_new[:, sl],
                                        es_v, None, MULT)
                    else:
                        V.tensor_mul(recS_new[:, sl], rec_new[:, sl], es_sb[:, sl])
                    m1_ = wk.tile([P, HF], F32, tag=f"w3{hf}", bufs=1)
                    V.tensor_mul(m1_[:], B_t[:, sl], r_new[:, sl])
                    V.tensor_add(q[:, sl], m1_[:], A_t[:, sl])
                    tt_ = wk.tile([P, HF], F32, tag=f"w4{hf}", bufs=1)
                    V.tensor_mul(tt_[:], r_new[:, sl], q[:, sl])
                    s2_ = wk.tile([P, HF], F32, tag=f"w5{hf}", bufs=1)
                    V.tensor_mul(s2_[:], D_t[:, sl], tt_[:])
                    V.scalar_tensor_tensor(v[:, sl], s2_[:], -1.0, C_t[:, sl],
                                           MULT, ADD)
                    ynew = newy[hf]
                    V.tensor_mul(ynew[:], tt_[:], v[:, sl])
                    return ynew

                yA_next = ew_half("A", mmTA)
                if not last:
                    nextA = launch_ag("A", yA_next)

                # remaining groups complete -> half B
                mmTB = transpose_jblocks("B", list(range(JA, J)))
                yB_next = ew_half("B", mmTB)
                if not last:
                    nextB = launch_ag("B", yB_next)
                    yfullA, yfullB = nextA, nextB
                    un, xn, recS = q, v, recS_new
                r = r_new

            # ---- epilogue ----
            for qi in range(4):
                nc.sync.dma_start(
                    r_out[32 * qi:32 * (qi + 1), :],
                    r[32 * qi:32 * (qi + 1), :],
                )

    nc.compile()
    return nc


# ---------------------------------------------------------------------------
# host-side data marshalling
# ---------------------------------------------------------------------------

def _shard_state(v, c):
    """[B, N] float array -> core c state tile [128, 256] (f32)."""
    vs = np.asarray(v, np.float32)[:, c * NS:(c + 1) * NS]      # [32, 1024]
    return np.ascontiguousarray(
        vs.reshape(B, J, P).transpose(2, 1, 0).reshape(P, F)
    )


def _shard_vec(v, c):
    """[N] float vector -> replicated core c tile [128, 256] (f32)."""
    vs = np.asarray(v, np.float32)[c * NS:(c + 1) * NS].reshape(J, P)  # [j, p]
    t = vs.T[:, :, None]                                        # [p, j, 1]
    return np.ascontiguousarray(np.broadcast_to(t, (P, J, B)).reshape(P, F))


def _shard_w(Wab, c):
    """Wab [N, N] -> core c weight tiles [64, 128, 1024] fp16.

    w[t, p, n] = Wab[c*1024 + n, t*128 + p]
    """
    wt = np.asarray(Wab, np.float32)[c * NS:(c + 1) * NS, :].T  # [8192, 1024]
    return np.ascontiguousarray(wt.astype(np.float16).reshape(T, P, NS))


def _unshard_out(tiles):
    """list of 8 [128, 256] tiles -> [32, 8192] f32."""
    out = np.empty((B, N), np.float32)
    for c, tl in enumerate(tiles):
        out[:, c * NS:(c + 1) * NS] = (
            np.asarray(tl, np.float32).reshape(P, J, B).transpose(2, 1, 0)
            .reshape(B, NS)
        )
    return out


def make_in_maps(rates, rec_input, ff_input, Wab, u_stp, x_stp,
                 exp_dt_tau, dt_tau, exp_dt_tau_syn, dt_tau_syn):
    recs_full = (np.asarray(exp_dt_tau_syn, np.float32)[None, :]
                 * np.asarray(rec_input, np.float32))
    in_maps = []
    for c in range(NCORES):
        in_maps.append({
            "w": _shard_w(Wab, c),
            "r0": _shard_state(rates, c),
            "recs0": _shard_state(recs_full, c),
            "u0": _shard_state(u_stp, c),
            "x0": _shard_state(x_stp, c),
            "ff": _shard_state(ff_input, c),
            "es": _shard_vec(exp_dt_tau_syn, c),
            "ds": _shard_vec(dt_tau_syn, c),
            "e": _shard_vec(exp_dt_tau, c),
            "dt": _shard_vec(dt_tau, c),
        })
    return in_maps


_PROGRAM_CACHE = {}


def _uniform_val(v):
    v = np.asarray(v, np.float32)
    return float(v.flat[0]) if np.all(v == v.flat[0]) else None


def _get_program(n_steps, uni):
    key = (n_steps, uni)
    if key not in _PROGRAM_CACHE:
        _PROGRAM_CACHE[key] = build_program(n_steps, uni=uni)
    return _PROGRAM_CACHE[key]


def run(trace=False, tmpdir=None, **inputs):
    n_steps = int(inputs.pop("n_steps"))
    uni = (_uniform_val(inputs["exp_dt_tau_syn"]),
           _uniform_val(inputs["dt_tau_syn"]),
           _uniform_val(inputs["exp_dt_tau"]),
           _uniform_val(inputs["dt_tau"]))
    nc = _get_program(n_steps, uni)
    in_maps = make_in_maps(**inputs)
    res = bass_utils.run_bass_kernel_spmd(
        nc, in_maps, core_ids=list(range(NCORES)), trace=trace, tmpdir=tmpdir
    )
    out = _unshard_out([m["r_out"] for m in res.results])
    return out, res


def kernel(**inputs):
    out, _ = run(**inputs)
    return out



# revision 11
# speedup vs baseline: 1.0972x; 1.0972x over previous
"""Trainium2 Bass kernel for the recurrent STP network (nn_Network_20109036880204).

Strategy (v5): tensor-parallel over the output-neuron dim across 8 NeuronCores,
with the per-step matmul in fp8 DoubleRow mode (2 fp8 weights per PE cell,
virtual contraction 256) to halve the moving-operand cycles.

  - Each core owns a 1024-neuron shard: W_c = Wab[c*1024:(c+1)*1024, :]^T,
    stored fp8e4 (x64 scaled) resident in SBUF as 64 K-tiles [128, 1024].
  - All [B, N] state tensors live in SBUF in "state layout": tile [128, 256]
    with  tile[p, j*32 + b] = state[b, n = c*1024 + j*128 + p].
  - y = u'*x'*r is exchanged in fp8e4 (x32 scaled) via two AllGathers per
    step (halves A = state cols 0..127, B = 128..255); the 1/(32*64) is
    folded into the dt_tau_syn multiply.
  - Matmul: per K-tile pair (t, t+1) one DoubleRow matmul contracts 256
    rows: lhsT = y[128, 2, 32] (3D AP over the gathered fp8 y), rhs =
    W[128, 2, 512] (3D AP over the resident weights, K-tile stride NS).
    Phase A accumulates output cols 0..511 (32 pairs), phase B cols
    512..1023, so half A's transposes + elementwise chain + AllGather fly
    under phase B's matmuls.
  - Next step's matmuls consume A-half-sourced K-pairs first so AG_B can
    land late; the gathered y is DMA'd in 3 chunks (c0, c1, c2-7) so the
    first matmuls ungate as soon as the first chunk lands.
  - The elementwise recurrence carries rf = es*rec + ff (instead of rec),
    which shortens the mm->y critical chain to 9 DVE ops per half.
"""

import sys

for _p in ("/opt/trn_rl_repo", "/root/.axon_site/_ro/trn_rl_repo"):
    if _p not in sys.path:
        sys.path.append(_p)

import ml_dtypes
import numpy as np

import concourse.bass as bass
import concourse.bacc as bacc
import concourse.mybir as mybir
import concourse.tile as tile
from concourse import bass_utils, masks

# problem constants
NCORES = 8
B = 32
N = 8192
NS = N // NCORES          # 1024 neurons per core
P = 128
J = NS // P               # 8 local K-tiles per core
T = N // P                # 64 K-tiles total
F = J * B                 # 256 = free size of a state tile
HW_ = 128                 # state-free width of a half (4 j-blocks)
GW = 512                  # output columns per psum group (A: 0..511, B: rest)

DT = 0.01
USE = 0.03
TAU_FAC = 1.0
TAU_REC = 0.25
C1 = DT / TAU_FAC         # 0.01
C0 = DT * USE / TAU_FAC   # 3e-4
A1 = USE * DT             # 3e-4
C2 = DT / TAU_REC         # 0.04

F32 = mybir.dt.float32
F16 = mybir.dt.float16
F8 = mybir.dt.float8e4
YSCALE = 32.0             # y is exchanged as fp8e4 * 32
WSCALE = 64.0             # W is resident as fp8e4 * 64
MULT = mybir.AluOpType.mult
ADD = mybir.AluOpType.add
MAX = mybir.AluOpType.max
DR = mybir.MatmulPerfMode.DoubleRow

# K-tile pairs (t, t+1): pair t covers neurons of j-blocks (t%8, t%8+1).
A_PAIRS = [t for t in range(T) if t % J in (0, 2)]   # y from half A
B_PAIRS = [t for t in range(T) if t % J in (4, 6)]   # y from half B
PAIR_ORDER = A_PAIRS + B_PAIRS                        # 32 pairs


def build_program(n_steps: int, uni=(None, None, None, None), n_dummy=16):
    """Build the SPMD Bass program (identical on all 8 cores)."""
    es_v, ds_v, e_v, dt_v = uni  # uniform values of the const vectors, or None

    nc = bacc.Bacc(
        "TRN2",
        target_bir_lowering=False,
        debug=False,
        num_devices=NCORES,
    )

    w_dram = nc.dram_tensor("w", [T, P, NS], F8, kind="ExternalInput")
    sd = {
        nm: nc.dram_tensor(nm, [P, F], F32, kind="ExternalInput")
        for nm in ["r0", "recs0", "u0", "x0", "ff", "es", "ds", "e", "dt"]
    }
    r_out = nc.dram_tensor("r_out", [P, F], F32, kind="ExternalOutput")

    with tile.TileContext(nc) as tc:
        with (
            tc.tile_pool(name="wpool", bufs=1) as wpool,
            tc.tile_pool(name="cpool", bufs=1) as cpool,
            tc.tile_pool(name="spool", bufs=2) as spool,
            tc.tile_pool(name="wk", bufs=2) as wk,
            tc.tile_pool(name="yp", bufs=2) as yp,
            tc.tile_pool(name="pmm", bufs=2, space="PSUM") as pmm,
            tc.tile_pool(name="pT", bufs=2, space="PSUM") as pT,
            tc.tile_pool(name="dp", bufs=3, space="DRAM") as dp,
        ):
            # ---- resident weights (fp8): 16 DMAs spread across queues ----
            w_sb = wpool.tile([P, T * NS], F8, tag="w")
            TB = 4  # K-tiles per DMA
            for i in range(T // TB):
                dst = w_sb[:, i * TB * NS:(i + 1) * TB * NS].rearrange(
                    "p (t n) -> p t n", t=TB
                )
                src = w_dram[i * TB:(i + 1) * TB, :, :].rearrange("t p n -> p t n")
                nc.sync.dma_start(dst, src)
            w_view = w_sb[:].rearrange("p (t n) -> p t n", t=T)

            # ---- constants / initial state ----
            ff_sb = cpool.tile([P, F], F32, tag="ff")
            es_sb = cpool.tile([P, F], F32, tag="es")
            ds_sb = cpool.tile([P, F], F32, tag="ds")
            e_sb = cpool.tile([P, F], F32, tag="e")
            dt_sb = cpool.tile([P, F], F32, tag="dt")
            ident = cpool.tile([B, B], F32, tag="ident")
            for t_, nm in [(ff_sb, "ff"), (es_sb, "es"), (ds_sb, "ds"),
                           (e_sb, "e"), (dt_sb, "dt")]:
                nc.sync.dma_start(t_[:], sd[nm][:])
            masks.make_identity(nc, ident[:])

            r = spool.tile([P, F], F32, tag="r")
            recS = spool.tile([P, F], F32, tag="recS")
            u0_sb = wk.tile([P, F], F32, tag="u0", bufs=1)
            x0_sb = wk.tile([P, F], F32, tag="x0", bufs=1)
            for t_, nm in [(r, "r0"), (recS, "recs0"), (u0_sb, "u0"),
                           (x0_sb, "x0")]:
                nc.sync.dma_start(t_[:], sd[nm][:])

            V = nc.vector

            # rf = es*rec + ff carry (recS0 from host is already es*rec0)
            rf = spool.tile([P, F], F32, tag="rf")
            V.tensor_add(rf[:], recS[:], ff_sb[:])
            # fme = ff - es*ff, so that rf' = es*h + fme (h = rec' + ff)
            fme = cpool.tile([P, F], F32, tag="fme")
            if es_v is not None:
                V.tensor_scalar(fme[:], ff_sb[:], 1.0 - es_v, None, MULT)
            else:
                tmp0 = wk.tile([P, F], F32, tag="tmp0", bufs=1)
                V.tensor_mul(tmp0[:], ff_sb[:], es_sb[:])
                V.tensor_sub(fme[:], ff_sb[:], tmp0[:])

            # ---- prologue: u1, x1, y0 from initial state ----
            s1 = wk.tile([P, F], F32, tag="t0", bufs=1)
            m = wk.tile([P, F], F32, tag="t1", bufs=1)
            s2 = wk.tile([P, F], F32, tag="t2", bufs=1)
            un = spool.tile([P, F], F32, tag="u")
            V.tensor_scalar(s1[:], u0_sb[:], 1.0 - C1, C0, MULT, ADD)
            V.tensor_mul(m[:], u0_sb[:], r[:])
            V.scalar_tensor_tensor(s2[:], r[:], A1, s1[:], MULT, ADD)
            V.scalar_tensor_tensor(un[:], m[:], -A1, s2[:], MULT, ADD)

            t2p = wk.tile([P, F], F32, tag="t3", bufs=1)
            t3p = wk.tile([P, F], F32, tag="t4", bufs=1)
            s4 = wk.tile([P, F], F32, tag="t5", bufs=1)
            xn = spool.tile([P, F], F32, tag="x")
            V.tensor_mul(t2p[:], x0_sb[:], r[:])
            V.tensor_mul(t3p[:], un[:], t2p[:])
            V.tensor_scalar(s4[:], x0_sb[:], 1.0 - C2, C2, MULT, ADD)
            V.scalar_tensor_tensor(xn[:], t3p[:], -DT, s4[:], MULT, ADD)

            w0 = wk.tile([P, F], F32, tag="t6", bufs=1)
            V.tensor_mul(w0[:], un[:], xn[:])
            yh = {}
            for hf, sl in (("A", slice(0, HW_)), ("B", slice(HW_, F))):
                yh[hf] = yp.tile([P, HW_], F8, tag=f"y{hf}",
                                 name=f"y{hf}_pro")
                V.scalar_tensor_tensor(yh[hf][:], w0[:, sl], YSCALE,
                                       r[:, sl], MULT, MULT)

            ag_counter = [0]

            def launch_ag(hf, ytile):
                """store y-half to DRAM, AllGather, DMA gathered chunks back."""
                k = ag_counter[0] = ag_counter[0] + 1
                ydr = dp.tile([P, HW_], F8, tag=f"ydr{hf}", name=f"ydr{hf}_{k}")
                nc.scalar.dma_start(ydr[:], ytile[:])
                yall = dp.tile([NCORES, P, HW_], F8, tag=f"yall{hf}",
                               name=f"yall{hf}_{k}", addr_space="Shared")
                nc.gpsimd.collective_compute(
                    "AllGather",
                    mybir.AluOpType.bypass,
                    replica_groups=[list(range(NCORES))],
                    ins=[ydr.opt()],
                    outs=[yall.opt()],
                )
                # 3 chunk tiles -> progressive ungating of the consumers
                y0 = yp.tile([P, HW_], F8, tag=f"yg0{hf}", name=f"yg0{hf}_{k}")
                y1 = yp.tile([P, HW_], F8, tag=f"yg1{hf}", name=f"yg1{hf}_{k}")
                yR = yp.tile([P, 6 * HW_], F8, tag=f"ygR{hf}",
                             name=f"ygR{hf}_{k}")
                nc.sync.dma_start(y0[:], yall[0, :, :])
                nc.sync.dma_start(y1[:], yall[1, :, :])
                nc.sync.dma_start(
                    yR[:].rearrange("p (c f) -> p c f", c=NCORES - 2),
                    yall[2:, :, :].rearrange("c p f -> p c f"),
                )
                return (y0, y1, yR)

            yfA = launch_ag("A", yh["A"])
            yfB = launch_ag("B", yh["B"])

            pdum = pmm.tile([B, 512], F32, tag="dummy", bufs=1,
                            name="pdum") if n_dummy else None

            # ---- main loop ----
            for it in range(n_steps):
                last = it == n_steps - 1

                def lhst_ap(t):
                    """y pair AP [128, 2, 32] for K-tile pair (t, t+1)."""
                    c, j = divmod(t, J)
                    yf = yfA if j < 4 else yfB
                    jj = j if j < 4 else j - 4
                    chunk = yf[c] if c < 2 else yf[2]
                    off = (0 if c < 2 else (c - 2) * HW_) + jj * B
                    return chunk[:, off:off + 2 * B].rearrange(
                        "p (k m) -> p k m", k=2)

                # precompute (overlaps matmuls on DVE)
                A_t = wk.tile([P, F], F32, tag="A", bufs=1)
                B_t = wk.tile([P, F], F32, tag="B", bufs=1)
                C_t = wk.tile([P, F], F32, tag="C", bufs=1)
                D_t = wk.tile([P, F], F32, tag="D", bufs=1)
                rE = wk.tile([P, F], F32, tag="rE", bufs=1)
                if not last:
                    V.tensor_scalar(A_t[:], un[:], 1.0 - C1, C0, MULT, ADD)
                    V.tensor_scalar(B_t[:], un[:], -A1, A1, MULT, ADD)
                    V.tensor_scalar(C_t[:], xn[:], 1.0 - C2, C2, MULT, ADD)
                    V.tensor_scalar(D_t[:], xn[:], DT, None, MULT)
                if e_v is None:
                    V.tensor_mul(rE[:], r[:], e_sb[:])

                pm = {"A": pmm.tile([B, GW], F32, tag="pmA", bufs=1,
                                    name=f"pmA_{it}"),
                      "B": pmm.tile([B, GW], F32, tag="pmB", bufs=1,
                                    name=f"pmB_{it}")}

                def emit_phase(hf):
                    lo = 0 if hf == "A" else GW
                    for i, t in enumerate(PAIR_ORDER):
                        nc.tensor.matmul(
                            pm[hf][:],
                            lhsT=lhst_ap(t),
                            rhs=w_view[:, t:t + 2, lo:lo + GW],
                            start=(i == 0),
                            stop=(i == len(PAIR_ORDER) - 1),
                            perf_mode=DR,
                        )

                def transpose_half(hf):
                    """psum [32, 512] -> state-layout PSUM [128, 128]."""
                    mmT_ = pT.tile([P, HW_], F32, tag=f"mmT{hf}", bufs=1,
                                   name=f"mmT{hf}_{it}")
                    stage = wk.tile([B, GW], F32, tag=f"stage{hf}",
                                    bufs=1, name=f"stage{hf}_{it}")
                    nc.scalar.copy(stage[:, :256], pm[hf][:, :256])
                    nc.scalar.copy(stage[:, 256:], pm[hf][:, 256:])
                    for jl in range(4):
                        nc.tensor.transpose(
                            mmT_[:, jl * B:(jl + 1) * B],
                            stage[:, jl * P:(jl + 1) * P],
                            ident[:],
                        )
                    return mmT_

                # dummy matmuls fill the AllGather wait at the step head,
                # keeping the PE HAM clock at full rate
                if n_dummy and it > 0:
                    for dk in range(n_dummy):
                        nc.tensor.matmul(
                            pdum[:], lhsT=w_sb[:, :B], rhs=w_sb[:, :512],
                            start=True, stop=True,
                        )

                emit_phase("A")
                mmTA = transpose_half("A")
                emit_phase("B")

                r_new = spool.tile([P, F], F32, tag="r")
                rf_new = spool.tile([P, F], F32, tag="rf")
                q = spool.tile([P, F], F32, tag="u")
                v = spool.tile([P, F], F32, tag="x")
                newy = {"A": yp.tile([P, HW_], F8, tag="yA", name=f"yA_{it}"),
                        "B": yp.tile([P, HW_], F8, tag="yB", name=f"yB_{it}")}

                def ew_half(hf, mmT_half):
                    sl = slice(0, HW_) if hf == "A" else slice(HW_, F)
                    # critical chain: mm -> y
                    h_ = wk.tile([P, HW_], F32, tag=f"w1{hf}", bufs=1)
                    if ds_v is not None:
                        V.scalar_tensor_tensor(h_[:], mmT_half[:],
                                               ds_v / (YSCALE * WSCALE),
                                               rf[:, sl], MULT, ADD)
                    else:
                        tmp = wk.tile([P, HW_], F32, tag=f"w0{hf}", bufs=1)
                        V.tensor_mul(tmp[:], mmT_half[:], ds_sb[:, sl])
                        V.scalar_tensor_tensor(h_[:], tmp[:],
                                               1.0 / (YSCALE * WSCALE),
                                               rf[:, sl], MULT, ADD)
                    dr_ = wk.tile([P, HW_], F32, tag=f"w2{hf}", bufs=1)
                    if dt_v is not None:
                        V.tensor_scalar(dr_[:], h_[:], 0.0, dt_v, MAX, MULT)
                    else:
                        V.scalar_tensor_tensor(dr_[:], h_[:], 0.0, dt_sb[:, sl],
                                               MAX, MULT)
                    if e_v is not None:
                        V.scalar_tensor_tensor(r_new[:, sl], r[:, sl], e_v,
                                               dr_[:], MULT, ADD)
                    else:
                        V.tensor_add(r_new[:, sl], dr_[:], rE[:, sl])
                    if last:
                        return None
                    m1_ = wk.tile([P, HW_], F32, tag=f"w3{hf}", bufs=1)
                    V.tensor_mul(m1_[:], B_t[:, sl], r_new[:, sl])
                    V.tensor_add(q[:, sl], m1_[:], A_t[:, sl])
                    tt_ = wk.tile([P, HW_], F32, tag=f"w4{hf}", bufs=1)
                    V.tensor_mul(tt_[:], r_new[:, sl], q[:, sl])
                    s2_ = wk.tile([P, HW_], F32, tag=f"w5{hf}", bufs=1)
                    V.tensor_mul(s2_[:], D_t[:, sl], tt_[:])
                    V.scalar_tensor_tensor(v[:, sl], s2_[:], -1.0, C_t[:, sl],
                                           MULT, ADD)
                    ynew = newy[hf]
                    V.scalar_tensor_tensor(ynew[:], tt_[:], YSCALE,
                                           v[:, sl], MULT, MULT)
                    # off critical path: rf' = es*h + fme
                    if es_v is not None:
                        V.scalar_tensor_tensor(rf_new[:, sl], h_[:], es_v,
                                               fme[:, sl], MULT, ADD)
                    else:
                        tmp2 = wk.tile([P, HW_], F32, tag=f"w6{hf}", bufs=1)
                        V.tensor_mul(tmp2[:], h_[:], es_sb[:, sl])
                        V.tensor_add(rf_new[:, sl], tmp2[:], fme[:, sl])
                    return ynew

                yA_next = ew_half("A", mmTA)
                if not last:
                    nextA = launch_ag("A", yA_next)

                mmTB = transpose_half("B")
                yB_next = ew_half("B", mmTB)
                if not last:
                    nextB = launch_ag("B", yB_next)
                    yfA, yfB = nextA, nextB
                    un, xn, rf = q, v, rf_new
                    yh = newy
                r = r_new

            # ---- epilogue ----
            for qi in range(4):
                nc.sync.dma_start(
                    r_out[32 * qi:32 * (qi + 1), :],
                    r[32 * qi:32 * (qi + 1), :],
                )

    nc.compile()
    return nc


# ---------------------------------------------------------------------------
# host-side data marshalling
# ---------------------------------------------------------------------------

def _shard_state(v, c):
    """[B, N] float array -> core c state tile [128, 256] (f32)."""
    vs = np.asarray(v, np.float32)[:, c * NS:(c + 1) * NS]      # [32, 1024]
    return np.ascontiguousarray(
        vs.reshape(B, J, P).transpose(2, 1, 0).reshape(P, F)
    )


def _shard_vec(v, c):
    """[N] float vector -> replicated core c tile [128, 256] (f32)."""
    vs = np.asarray(v, np.float32)[c * NS:(c + 1) * NS].reshape(J, P)  # [j, p]
    t = vs.T[:, :, None]                                        # [p, j, 1]
    return np.ascontiguousarray(np.broadcast_to(t, (P, J, B)).reshape(P, F))


def _shard_w(Wab, c):
    """Wab [N, N] -> core c weight tiles [64, 128, 1024] fp8e4 (x64).

    w[t, p, n] = Wab[c*1024 + n, t*128 + p] * 64
    """
    wt = np.asarray(Wab, np.float32)[c * NS:(c + 1) * NS, :].T  # [8192, 1024]
    return np.ascontiguousarray(
        (wt * WSCALE).astype(ml_dtypes.float8_e4m3fn).reshape(T, P, NS))


def _unshard_out(tiles):
    """list of 8 [128, 256] tiles -> [32, 8192] f32."""
    out = np.empty((B, N), np.float32)
    for c, tl in enumerate(tiles):
        out[:, c * NS:(c + 1) * NS] = (
            np.asarray(tl, np.float32).reshape(P, J, B).transpose(2, 1, 0)
            .reshape(B, NS)
        )
    return out


def make_in_maps(rates, rec_input, ff_input, Wab, u_stp, x_stp,
                 exp_dt_tau, dt_tau, exp_dt_tau_syn, dt_tau_syn):
    recs_full = (np.asarray(exp_dt_tau_syn, np.float32)[None, :]
                 * np.asarray(rec_input, np.float32))
    in_maps = []
    for c in range(NCORES):
        in_maps.append({
            "w": _shard_w(Wab, c),
            "r0": _shard_state(rates, c),
            "recs0": _shard_state(recs_full, c),
            "u0": _shard_state(u_stp, c),
            "x0": _shard_state(x_stp, c),
            "ff": _shard_state(ff_input, c),
            "es": _shard_vec(exp_dt_tau_syn, c),
            "ds": _shard_vec(dt_tau_syn, c),
            "e": _shard_vec(exp_dt_tau, c),
            "dt": _shard_vec(dt_tau, c),
        })
    return in_maps


_PROGRAM_CACHE = {}


def _uniform_val(v):
    v = np.asarray(v, np.float32)
    return float(v.flat[0]) if np.all(v == v.flat[0]) else None


def _get_program(n_steps, uni):
    key = (n_steps, uni)
    if key not in _PROGRAM_CACHE:
        _PROGRAM_CACHE[key] = build_program(n_steps, uni=uni)
    return _PROGRAM_CACHE[key]


def run(trace=False, tmpdir=None, **inputs):
    n_steps = int(inputs.pop("n_steps"))
    uni = (_uniform_val(inputs["exp_dt_tau_syn"]),
           _uniform_val(inputs["dt_tau_syn"]),
           _uniform_val(inputs["exp_dt_tau"]),
           _uniform_val(inputs["dt_tau"]))
    nc = _get_program(n_steps, uni)
    in_maps = make_in_maps(**inputs)
    res = bass_utils.run_bass_kernel_spmd(
        nc, in_maps, core_ids=list(range(NCORES)), trace=trace, tmpdir=tmpdir
    )
    out = _unshard_out([m["r_out"] for m in res.results])
    return out, res


def kernel(**inputs):
    out, _ = run(**inputs)
    return out


# revision 12
# speedup vs baseline: 1.2594x; 1.1478x over previous
"""Trainium2 Bass kernel for the recurrent STP network (nn_Network_20109036880204).

Strategy (v5): tensor-parallel over the output-neuron dim across 8 NeuronCores,
with the per-step matmul in fp8 DoubleRow mode (2 fp8 weights per PE cell,
virtual contraction 256) to halve the moving-operand cycles.

  - Each core owns a 1024-neuron shard: W_c = Wab[c*1024:(c+1)*1024, :]^T,
    stored fp8e4 (x64 scaled) resident in SBUF as 64 K-tiles [128, 1024].
  - All [B, N] state tensors live in SBUF in "state layout": tile [128, 256]
    with  tile[p, j*32 + b] = state[b, n = c*1024 + j*128 + p].
  - y = u'*x'*r is exchanged in fp8e4 (x32 scaled) via two AllGathers per
    step (halves A = state cols 0..127, B = 128..255); the 1/(32*64) is
    folded into the dt_tau_syn multiply.
  - Matmul: per K-tile pair (t, t+1) one DoubleRow matmul contracts 256
    rows: lhsT = y[128, 2, 32] (3D AP over the gathered fp8 y), rhs =
    W[128, 2, 512] (3D AP over the resident weights, K-tile stride NS).
    Phase A accumulates output cols 0..511 (32 pairs), phase B cols
    512..1023, so half A's transposes + elementwise chain + AllGather fly
    under phase B's matmuls.
  - Next step's matmuls consume A-half-sourced K-pairs first so AG_B can
    land late; the gathered y is DMA'd in 3 chunks (c0, c1, c2-7) so the
    first matmuls ungate as soon as the first chunk lands.
  - The elementwise recurrence carries rf = es*rec + ff (instead of rec),
    which shortens the mm->y critical chain to 9 DVE ops per half.
"""

import sys

for _p in ("/opt/trn_rl_repo", "/root/.axon_site/_ro/trn_rl_repo"):
    if _p not in sys.path:
        sys.path.append(_p)

import ml_dtypes
import numpy as np

import concourse.bass as bass
import concourse.bacc as bacc
import concourse.mybir as mybir
import concourse.tile as tile
from concourse import bass_utils, masks

# problem constants
NCORES = 8
B = 32
N = 8192
NS = N // NCORES          # 1024 neurons per core
P = 128
J = NS // P               # 8 local K-tiles per core
T = N // P                # 64 K-tiles total
F = J * B                 # 256 = free size of a state tile
HW_ = 128                 # state-free width of a half (4 j-blocks)
GW = 256                  # output columns per PE column-group

DT = 0.01
USE = 0.03
TAU_FAC = 1.0
TAU_REC = 0.25
C1 = DT / TAU_FAC         # 0.01
C0 = DT * USE / TAU_FAC   # 3e-4
A1 = USE * DT             # 3e-4
C2 = DT / TAU_REC         # 0.04

F32 = mybir.dt.float32
F16 = mybir.dt.float16
F8 = mybir.dt.float8e4
YSCALE = 32.0             # y is exchanged as fp8e4 * 32
WSCALE = 64.0             # W is resident as fp8e4 * 64
MULT = mybir.AluOpType.mult
ADD = mybir.AluOpType.add
MAX = mybir.AluOpType.max
DR = mybir.MatmulPerfMode.DoubleRow

# K-tile halves: tile t holds neurons n = c*1024 + j*128 + [0,128), j = t%8.
A_TILES = [t for t in range(T) if t % J < 4]
B_TILES = [t for t in range(T) if t % J >= 4]
KORDER = A_TILES + B_TILES


def build_program(n_steps: int, uni=(None, None, None, None), n_dummy=16):
    """Build the SPMD Bass program (identical on all 8 cores)."""
    es_v, ds_v, e_v, dt_v = uni  # uniform values of the const vectors, or None

    nc = bacc.Bacc(
        "TRN2",
        target_bir_lowering=False,
        debug=False,
        num_devices=NCORES,
    )

    w_dram = nc.dram_tensor("w", [T, P, NS], F16, kind="ExternalInput")
    sd = {
        nm: nc.dram_tensor(nm, [P, F], F32, kind="ExternalInput")
        for nm in ["r0", "recs0", "u0", "x0", "ff", "es", "ds", "e", "dt"]
    }
    r_out = nc.dram_tensor("r_out", [P, F], F32, kind="ExternalOutput")

    with tile.TileContext(nc) as tc:
        with (
            tc.tile_pool(name="wpool", bufs=1) as wpool,
            tc.tile_pool(name="cpool", bufs=1) as cpool,
            tc.tile_pool(name="spool", bufs=2) as spool,
            tc.tile_pool(name="wk", bufs=2) as wk,
            tc.tile_pool(name="yp", bufs=2) as yp,
            tc.tile_pool(name="pmm", bufs=2, space="PSUM") as pmm,
            tc.tile_pool(name="pT", bufs=2, space="PSUM") as pT,
            tc.tile_pool(name="dp", bufs=3, space="DRAM") as dp,
        ):
            # ---- resident weights (fp8): 16 DMAs spread across queues ----
            w_sb = wpool.tile([P, T * NS], F16, tag="w")
            TB = 4  # K-tiles per DMA
            for i in range(T // TB):
                dst = w_sb[:, i * TB * NS:(i + 1) * TB * NS].rearrange(
                    "p (t n) -> p t n", t=TB
                )
                src = w_dram[i * TB:(i + 1) * TB, :, :].rearrange("t p n -> p t n")
                nc.sync.dma_start(dst, src)

            # ---- constants / initial state ----
            ff_sb = cpool.tile([P, F], F32, tag="ff")
            es_sb = cpool.tile([P, F], F32, tag="es")
            ds_sb = cpool.tile([P, F], F32, tag="ds")
            e_sb = cpool.tile([P, F], F32, tag="e")
            dt_sb = cpool.tile([P, F], F32, tag="dt")
            identF = cpool.tile([P, B], F32, tag="identF")
            for t_, nm in [(ff_sb, "ff"), (es_sb, "es"), (ds_sb, "ds"),
                           (e_sb, "e"), (dt_sb, "dt")]:
                nc.sync.dma_start(t_[:], sd[nm][:])
            for g in range(4):
                masks.make_identity(nc, identF[32 * g:32 * (g + 1), :])

            r = spool.tile([P, F], F32, tag="r")
            recS = spool.tile([P, F], F32, tag="recS")
            u0_sb = wk.tile([P, F], F32, tag="u0", bufs=1)
            x0_sb = wk.tile([P, F], F32, tag="x0", bufs=1)
            for t_, nm in [(r, "r0"), (recS, "recs0"), (u0_sb, "u0"),
                           (x0_sb, "x0")]:
                nc.sync.dma_start(t_[:], sd[nm][:])

            V = nc.vector

            # rf = es*rec + ff carry (recS0 from host is already es*rec0)
            rf = spool.tile([P, F], F32, tag="rf")
            V.tensor_add(rf[:], recS[:], ff_sb[:])
            # fme = ff - es*ff, so that rf' = es*h + fme (h = rec' + ff)
            fme = cpool.tile([P, F], F32, tag="fme")
            if es_v is not None:
                V.tensor_scalar(fme[:], ff_sb[:], 1.0 - es_v, None, MULT)
            else:
                tmp0 = wk.tile([P, F], F32, tag="tmp0", bufs=1)
                V.tensor_mul(tmp0[:], ff_sb[:], es_sb[:])
                V.tensor_sub(fme[:], ff_sb[:], tmp0[:])

            # ---- prologue: u1, x1, y0 from initial state ----
            s1 = wk.tile([P, F], F32, tag="t0", bufs=1)
            m = wk.tile([P, F], F32, tag="t1", bufs=1)
            s2 = wk.tile([P, F], F32, tag="t2", bufs=1)
            un = spool.tile([P, F], F32, tag="u")
            V.tensor_scalar(s1[:], u0_sb[:], 1.0 - C1, C0, MULT, ADD)
            V.tensor_mul(m[:], u0_sb[:], r[:])
            V.scalar_tensor_tensor(s2[:], r[:], A1, s1[:], MULT, ADD)
            V.scalar_tensor_tensor(un[:], m[:], -A1, s2[:], MULT, ADD)

            t2p = wk.tile([P, F], F32, tag="t3", bufs=1)
            t3p = wk.tile([P, F], F32, tag="t4", bufs=1)
            s4 = wk.tile([P, F], F32, tag="t5", bufs=1)
            xn = spool.tile([P, F], F32, tag="x")
            V.tensor_mul(t2p[:], x0_sb[:], r[:])
            V.tensor_mul(t3p[:], un[:], t2p[:])
            V.tensor_scalar(s4[:], x0_sb[:], 1.0 - C2, C2, MULT, ADD)
            V.scalar_tensor_tensor(xn[:], t3p[:], -DT, s4[:], MULT, ADD)

            w0 = wk.tile([P, F], F32, tag="t6", bufs=1)
            V.tensor_mul(w0[:], un[:], xn[:])
            yh = {}
            for hf, sl in (("A", slice(0, HW_)), ("B", slice(HW_, F))):
                yh[hf] = yp.tile([P, HW_], F8, tag=f"y{hf}",
                                 name=f"y{hf}_pro")
                V.scalar_tensor_tensor(yh[hf][:], w0[:, sl], YSCALE,
                                       r[:, sl], MULT, MULT)

            ag_counter = [0]

            def launch_ag(hf, ytile):
                """store y-half to DRAM, AllGather, DMA gathered chunks back."""
                k = ag_counter[0] = ag_counter[0] + 1
                ydr = dp.tile([P, HW_], F8, tag=f"ydr{hf}", name=f"ydr{hf}_{k}")
                nc.scalar.dma_start(ydr[:], ytile[:])
                yall = dp.tile([NCORES, P, HW_], F8, tag=f"yall{hf}",
                               name=f"yall{hf}_{k}", addr_space="Shared")
                nc.gpsimd.collective_compute(
                    "AllGather",
                    mybir.AluOpType.bypass,
                    replica_groups=[list(range(NCORES))],
                    ins=[ydr.opt()],
                    outs=[yall.opt()],
                )
                # 3 chunk tiles -> progressive ungating of the consumers
                y0 = yp.tile([P, HW_], F8, tag=f"yg0{hf}", name=f"yg0{hf}_{k}")
                y1 = yp.tile([P, HW_], F8, tag=f"yg1{hf}", name=f"yg1{hf}_{k}")
                yR = yp.tile([P, 6 * HW_], F8, tag=f"ygR{hf}",
                             name=f"ygR{hf}_{k}")
                nc.sync.dma_start(y0[:], yall[0, :, :])
                nc.sync.dma_start(y1[:], yall[1, :, :])
                nc.sync.dma_start(
                    yR[:].rearrange("p (c f) -> p c f", c=NCORES - 2),
                    yall[2:, :, :].rearrange("c p f -> p c f"),
                )
                return (y0, y1, yR)

            yfA = launch_ag("A", yh["A"])
            yfB = launch_ag("B", yh["B"])

            pdum = pmm.tile([B, 512], F32, tag="dummy", bufs=1,
                            name="pdum") if n_dummy else None

            # ---- main loop ----
            for it in range(n_steps):
                last = it == n_steps - 1

                def lhst_ap(t):
                    """y K-tile AP [128, 32]."""
                    c, j = divmod(t, J)
                    yf = yfA if j < 4 else yfB
                    jj = j if j < 4 else j - 4
                    chunk = yf[c] if c < 2 else yf[2]
                    off = (0 if c < 2 else (c - 2) * HW_) + jj * B
                    return chunk[:, off:off + B]

                # precompute (overlaps matmuls on DVE)
                A_t = wk.tile([P, F], F32, tag="A", bufs=1)
                B_t = wk.tile([P, F], F32, tag="B", bufs=1)
                C_t = wk.tile([P, F], F32, tag="C", bufs=1)
                D_t = wk.tile([P, F], F32, tag="D", bufs=1)
                rE = wk.tile([P, F], F32, tag="rE", bufs=1)
                if not last:
                    V.tensor_scalar(A_t[:], un[:], 1.0 - C1, C0, MULT, ADD)
                    V.tensor_scalar(B_t[:], un[:], -A1, A1, MULT, ADD)
                    V.tensor_scalar(C_t[:], xn[:], 1.0 - C2, C2, MULT, ADD)
                    V.tensor_scalar(D_t[:], xn[:], DT, None, MULT)
                if e_v is None:
                    V.tensor_mul(rE[:], r[:], e_sb[:])

                # column-group PSUM tiles: group g accumulates at
                # partitions [32g, 32g+32), each in its own 2 KiB bank
                # (matmul start=True claims a whole zero region).
                pg = [pmm.tile([P, 512], F32, tag=f"pg{g}", bufs=1,
                               name=f"pg{g}_{it}") for g in range(4)]

                def emit_waves(groups, tiles, base_idx):
                    for k_, t in enumerate(tiles):
                        ki = base_idx + k_
                        for g in groups:
                            nc.tensor.matmul(
                                pg[g][32 * g:32 * (g + 1), :GW],
                                lhsT=lhst_ap(t),
                                rhs=w_sb[:, t * NS + g * GW:
                                         t * NS + (g + 1) * GW],
                                start=(ki == 0),
                                stop=(ki == T - 1),
                                tile_position=(0, 32 * g),
                            )

                def transpose_half(hf, groups):
                    """PSUM column-groups -> state-layout PSUM [128, 128]."""
                    mmT_ = pT.tile([P, HW_], F32, tag=f"mmT{hf}", bufs=1,
                                   name=f"mmT{hf}_{it}")
                    stage = wk.tile([P, GW], F32, tag=f"stage{hf}",
                                    bufs=1, name=f"stage{hf}_{it}")
                    for g in groups:
                        nc.scalar.copy(stage[32 * g:32 * (g + 1), :],
                                       pg[g][32 * g:32 * (g + 1), :GW])
                    for jl in range(4):
                        g = groups[jl // 2]
                        jj = jl % 2
                        nc.tensor.transpose(
                            mmT_[:, jl * B:(jl + 1) * B],
                            stage[32 * g:32 * (g + 1),
                                  jj * P:(jj + 1) * P],
                            identF[32 * g:32 * (g + 1), :],
                            tile_position=(32 * g, 0),
                        )
                    return mmT_

                # dummy matmuls fill the AllGather wait at the step end:
                # gated on a copy of the fresh y (ready right after ew_B)
                # so the scheduler cannot hoist them earlier.
                if n_dummy and it > 0:
                    dcp = wk.tile([P, B], F16, tag="dcp", bufs=1,
                                  name=f"dcp_{it}")
                    nc.scalar.copy(dcp[:], w_sb[:, :B])
                    for dk in range(n_dummy):
                        nc.tensor.matmul(
                            pdum[:], lhsT=dcp[:], rhs=w_sb[:, :512],
                            start=True, stop=True,
                        )

                emit_waves((0, 1), KORDER, 0)
                mmTA = transpose_half("A", (0, 1))
                emit_waves((2, 3), KORDER, 0)

                r_new = spool.tile([P, F], F32, tag="r")
                rf_new = spool.tile([P, F], F32, tag="rf")
                q = spool.tile([P, F], F32, tag="u")
                v = spool.tile([P, F], F32, tag="x")
                newy = {"A": yp.tile([P, HW_], F8, tag="yA", name=f"yA_{it}"),
                        "B": yp.tile([P, HW_], F8, tag="yB", name=f"yB_{it}")}

                def ew_half(hf, mmT_half):
                    sl = slice(0, HW_) if hf == "A" else slice(HW_, F)
                    # critical chain: mm -> y
                    h_ = wk.tile([P, HW_], F32, tag=f"w1{hf}", bufs=1)
                    if ds_v is not None:
                        V.scalar_tensor_tensor(h_[:], mmT_half[:],
                                               ds_v / YSCALE,
                                               rf[:, sl], MULT, ADD)
                    else:
                        tmp = wk.tile([P, HW_], F32, tag=f"w0{hf}", bufs=1)
                        V.tensor_mul(tmp[:], mmT_half[:], ds_sb[:, sl])
                        V.scalar_tensor_tensor(h_[:], tmp[:],
                                               1.0 / YSCALE,
                                               rf[:, sl], MULT, ADD)
                    dr_ = wk.tile([P, HW_], F32, tag=f"w2{hf}", bufs=1)
                    if dt_v is not None:
                        V.tensor_scalar(dr_[:], h_[:], 0.0, dt_v, MAX, MULT)
                    else:
                        V.scalar_tensor_tensor(dr_[:], h_[:], 0.0, dt_sb[:, sl],
                                               MAX, MULT)
                    if e_v is not None:
                        V.scalar_tensor_tensor(r_new[:, sl], r[:, sl], e_v,
                                               dr_[:], MULT, ADD)
                    else:
                        V.tensor_add(r_new[:, sl], dr_[:], rE[:, sl])
                    if last:
                        return None
                    m1_ = wk.tile([P, HW_], F32, tag=f"w3{hf}", bufs=1)
                    V.tensor_mul(m1_[:], B_t[:, sl], r_new[:, sl])
                    V.tensor_add(q[:, sl], m1_[:], A_t[:, sl])
                    tt_ = wk.tile([P, HW_], F32, tag=f"w4{hf}", bufs=1)
                    V.tensor_mul(tt_[:], r_new[:, sl], q[:, sl])
                    s2_ = wk.tile([P, HW_], F32, tag=f"w5{hf}", bufs=1)
                    V.tensor_mul(s2_[:], D_t[:, sl], tt_[:])
                    V.scalar_tensor_tensor(v[:, sl], s2_[:], -1.0, C_t[:, sl],
                                           MULT, ADD)
                    ynew = newy[hf]
                    V.scalar_tensor_tensor(ynew[:], tt_[:], YSCALE,
                                           v[:, sl], MULT, MULT)
                    # off critical path: rf' = es*h + fme
                    if es_v is not None:
                        V.scalar_tensor_tensor(rf_new[:, sl], h_[:], es_v,
                                               fme[:, sl], MULT, ADD)
                    else:
                        tmp2 = wk.tile([P, HW_], F32, tag=f"w6{hf}", bufs=1)
                        V.tensor_mul(tmp2[:], h_[:], es_sb[:, sl])
                        V.tensor_add(rf_new[:, sl], tmp2[:], fme[:, sl])
                    return ynew

                yA_next = ew_half("A", mmTA)
                if not last:
                    nextA = launch_ag("A", yA_next)

                mmTB = transpose_half("B", (2, 3))
                yB_next = ew_half("B", mmTB)
                if not last:
                    nextB = launch_ag("B", yB_next)
                    yfA, yfB = nextA, nextB
                    un, xn, rf = q, v, rf_new
                    yh = newy
                r = r_new

            # ---- epilogue ----
            for qi in range(4):
                nc.sync.dma_start(
                    r_out[32 * qi:32 * (qi + 1), :],
                    r[32 * qi:32 * (qi + 1), :],
                )

    nc.compile()
    return nc


# ---------------------------------------------------------------------------
# host-side data marshalling
# ---------------------------------------------------------------------------

def _shard_state(v, c):
    """[B, N] float array -> core c state tile [128, 256] (f32)."""
    vs = np.asarray(v, np.float32)[:, c * NS:(c + 1) * NS]      # [32, 1024]
    return np.ascontiguousarray(
        vs.reshape(B, J, P).transpose(2, 1, 0).reshape(P, F)
    )


def _shard_vec(v, c):
    """[N] float vector -> replicated core c tile [128, 256] (f32)."""
    vs = np.asarray(v, np.float32)[c * NS:(c + 1) * NS].reshape(J, P)  # [j, p]
    t = vs.T[:, :, None]                                        # [p, j, 1]
    return np.ascontiguousarray(np.broadcast_to(t, (P, J, B)).reshape(P, F))


def _shard_w(Wab, c):
    """Wab [N, N] -> core c weight tiles [64, 128, 1024] fp16.

    w[t, p, n] = Wab[c*1024 + n, t*128 + p]
    """
    wt = np.asarray(Wab, np.float32)[c * NS:(c + 1) * NS, :].T  # [8192, 1024]
    return np.ascontiguousarray(wt.astype(np.float16).reshape(T, P, NS))


def _unshard_out(tiles):
    """list of 8 [128, 256] tiles -> [32, 8192] f32."""
    out = np.empty((B, N), np.float32)
    for c, tl in enumerate(tiles):
        out[:, c * NS:(c + 1) * NS] = (
            np.asarray(tl, np.float32).reshape(P, J, B).transpose(2, 1, 0)
            .reshape(B, NS)
        )
    return out


def make_in_maps(rates, rec_input, ff_input, Wab, u_stp, x_stp,
                 exp_dt_tau, dt_tau, exp_dt_tau_syn, dt_tau_syn):
    recs_full = (np.asarray(exp_dt_tau_syn, np.float32)[None, :]
                 * np.asarray(rec_input, np.float32))
    in_maps = []
    for c in range(NCORES):
        in_maps.append({
            "w": _shard_w(Wab, c),
            "r0": _shard_state(rates, c),
            "recs0": _shard_state(recs_full, c),
            "u0": _shard_state(u_stp, c),
            "x0": _shard_state(x_stp, c),
            "ff": _shard_state(ff_input, c),
            "es": _shard_vec(exp_dt_tau_syn, c),
            "ds": _shard_vec(dt_tau_syn, c),
            "e": _shard_vec(exp_dt_tau, c),
            "dt": _shard_vec(dt_tau, c),
        })
    return in_maps


_PROGRAM_CACHE = {}


def _uniform_val(v):
    v = np.asarray(v, np.float32)
    return float(v.flat[0]) if np.all(v == v.flat[0]) else None


def _get_program(n_steps, uni):
    key = (n_steps, uni)
    if key not in _PROGRAM_CACHE:
        _PROGRAM_CACHE[key] = build_program(n_steps, uni=uni)
    return _PROGRAM_CACHE[key]


def run(trace=False, tmpdir=None, **inputs):
    n_steps = int(inputs.pop("n_steps"))
    uni = (_uniform_val(inputs["exp_dt_tau_syn"]),
           _uniform_val(inputs["dt_tau_syn"]),
           _uniform_val(inputs["exp_dt_tau"]),
           _uniform_val(inputs["dt_tau"]))
    nc = _get_program(n_steps, uni)
    in_maps = make_in_maps(**inputs)
    res = bass_utils.run_bass_kernel_spmd(
        nc, in_maps, core_ids=list(range(NCORES)), trace=trace, tmpdir=tmpdir
    )
    out = _unshard_out([m["r_out"] for m in res.results])
    return out, res


def kernel(**inputs):
    out, _ = run(**inputs)
    return out


# revision 13
# speedup vs baseline: 1.3203x; 1.0484x over previous
"""Trainium2 Bass kernel for the recurrent STP network (nn_Network_20109036880204).

Strategy (v5): tensor-parallel over the output-neuron dim across 8 NeuronCores,
with the per-step matmul in fp8 DoubleRow mode (2 fp8 weights per PE cell,
virtual contraction 256) to halve the moving-operand cycles.

  - Each core owns a 1024-neuron shard: W_c = Wab[c*1024:(c+1)*1024, :]^T,
    stored fp8e4 (x64 scaled) resident in SBUF as 64 K-tiles [128, 1024].
  - All [B, N] state tensors live in SBUF in "state layout": tile [128, 256]
    with  tile[p, j*32 + b] = state[b, n = c*1024 + j*128 + p].
  - y = u'*x'*r is exchanged in fp8e4 (x32 scaled) via two AllGathers per
    step (halves A = state cols 0..127, B = 128..255); the 1/(32*64) is
    folded into the dt_tau_syn multiply.
  - Matmul: per K-tile pair (t, t+1) one DoubleRow matmul contracts 256
    rows: lhsT = y[128, 2, 32] (3D AP over the gathered fp8 y), rhs =
    W[128, 2, 512] (3D AP over the resident weights, K-tile stride NS).
    Phase A accumulates output cols 0..511 (32 pairs), phase B cols
    512..1023, so half A's transposes + elementwise chain + AllGather fly
    under phase B's matmuls.
  - Next step's matmuls consume A-half-sourced K-pairs first so AG_B can
    land late; the gathered y is DMA'd in 3 chunks (c0, c1, c2-7) so the
    first matmuls ungate as soon as the first chunk lands.
  - The elementwise recurrence carries rf = es*rec + ff (instead of rec),
    which shortens the mm->y critical chain to 9 DVE ops per half.
"""

import sys

for _p in ("/opt/trn_rl_repo", "/root/.axon_site/_ro/trn_rl_repo"):
    if _p not in sys.path:
        sys.path.append(_p)

import ml_dtypes
import numpy as np

import concourse.bass as bass
import concourse.bacc as bacc
import concourse.mybir as mybir
import concourse.tile as tile
from concourse import bass_utils, masks

# problem constants
NCORES = 8
B = 32
N = 8192
NS = N // NCORES          # 1024 neurons per core
P = 128
J = NS // P               # 8 local K-tiles per core
T = N // P                # 64 K-tiles total
F = J * B                 # 256 = free size of a state tile
HW_ = 128                 # state-free width of a half (4 j-blocks)
GW = 256                  # output columns per PE column-group

DT = 0.01
USE = 0.03
TAU_FAC = 1.0
TAU_REC = 0.25
C1 = DT / TAU_FAC         # 0.01
C0 = DT * USE / TAU_FAC   # 3e-4
A1 = USE * DT             # 3e-4
C2 = DT / TAU_REC         # 0.04

F32 = mybir.dt.float32
F16 = mybir.dt.float16
F8 = mybir.dt.float8e4
YSCALE = 32.0             # y is exchanged as fp8e4 * 32
WSCALE = 64.0             # W is resident as fp8e4 * 64
MULT = mybir.AluOpType.mult
ADD = mybir.AluOpType.add
MAX = mybir.AluOpType.max
DR = mybir.MatmulPerfMode.DoubleRow

# K-tile halves: tile t holds neurons n = c*1024 + j*128 + [0,128), j = t%8.
A_TILES = [t for t in range(T) if t % J < 4]
B_TILES = [t for t in range(T) if t % J >= 4]
KORDER = A_TILES + B_TILES


def build_program(n_steps: int, uni=(None, None, None, None), n_dummy=12):
    """Build the SPMD Bass program (identical on all 8 cores)."""
    es_v, ds_v, e_v, dt_v = uni  # uniform values of the const vectors, or None

    nc = bacc.Bacc(
        "TRN2",
        target_bir_lowering=False,
        debug=False,
        num_devices=NCORES,
    )

    w_dram = nc.dram_tensor("w", [T, P, NS], F16, kind="ExternalInput")
    sd = {
        nm: nc.dram_tensor(nm, [P, F], F32, kind="ExternalInput")
        for nm in ["r0", "recs0", "u0", "x0", "ff", "es", "ds", "e", "dt"]
    }
    r_out = nc.dram_tensor("r_out", [P, F], F32, kind="ExternalOutput")

    with tile.TileContext(nc) as tc:
        with (
            tc.tile_pool(name="wpool", bufs=1) as wpool,
            tc.tile_pool(name="cpool", bufs=1) as cpool,
            tc.tile_pool(name="spool", bufs=2) as spool,
            tc.tile_pool(name="wk", bufs=2) as wk,
            tc.tile_pool(name="yp", bufs=2) as yp,
            tc.tile_pool(name="pmm", bufs=2, space="PSUM") as pmm,
            tc.tile_pool(name="pT", bufs=2, space="PSUM") as pT,
            tc.tile_pool(name="dp", bufs=3, space="DRAM") as dp,
        ):
            # ---- resident weights (fp8): 16 DMAs spread across queues ----
            w_sb = wpool.tile([P, T * NS], F16, tag="w")
            TB = 4  # K-tiles per DMA
            for i in range(T // TB):
                dst = w_sb[:, i * TB * NS:(i + 1) * TB * NS].rearrange(
                    "p (t n) -> p t n", t=TB
                )
                src = w_dram[i * TB:(i + 1) * TB, :, :].rearrange("t p n -> p t n")
                nc.sync.dma_start(dst, src)

            # ---- constants / initial state ----
            ff_sb = cpool.tile([P, F], F32, tag="ff")
            es_sb = cpool.tile([P, F], F32, tag="es")
            ds_sb = cpool.tile([P, F], F32, tag="ds")
            e_sb = cpool.tile([P, F], F32, tag="e")
            dt_sb = cpool.tile([P, F], F32, tag="dt")
            identF = cpool.tile([P, B], F32, tag="identF")
            for t_, nm in [(ff_sb, "ff"), (es_sb, "es"), (ds_sb, "ds"),
                           (e_sb, "e"), (dt_sb, "dt")]:
                nc.sync.dma_start(t_[:], sd[nm][:])
            for g in range(4):
                masks.make_identity(nc, identF[32 * g:32 * (g + 1), :])

            r = spool.tile([P, F], F32, tag="r")
            recS = spool.tile([P, F], F32, tag="recS")
            u0_sb = wk.tile([P, F], F32, tag="u0", bufs=1)
            x0_sb = wk.tile([P, F], F32, tag="x0", bufs=1)
            for t_, nm in [(r, "r0"), (recS, "recs0"), (u0_sb, "u0"),
                           (x0_sb, "x0")]:
                nc.sync.dma_start(t_[:], sd[nm][:])

            V = nc.vector

            # rf = es*rec + ff carry (recS0 from host is already es*rec0)
            rf = spool.tile([P, F], F32, tag="rf")
            V.tensor_add(rf[:], recS[:], ff_sb[:])
            # fme = ff - es*ff, so that rf' = es*h + fme (h = rec' + ff)
            fme = cpool.tile([P, F], F32, tag="fme")
            if es_v is not None:
                V.tensor_scalar(fme[:], ff_sb[:], 1.0 - es_v, None, MULT)
            else:
                tmp0 = wk.tile([P, F], F32, tag="tmp0", bufs=1)
                V.tensor_mul(tmp0[:], ff_sb[:], es_sb[:])
                V.tensor_sub(fme[:], ff_sb[:], tmp0[:])

            # ---- prologue: u1, x1, y0 from initial state ----
            s1 = wk.tile([P, F], F32, tag="t0", bufs=1)
            m = wk.tile([P, F], F32, tag="t1", bufs=1)
            s2 = wk.tile([P, F], F32, tag="t2", bufs=1)
            un = spool.tile([P, F], F32, tag="u")
            V.tensor_scalar(s1[:], u0_sb[:], 1.0 - C1, C0, MULT, ADD)
            V.tensor_mul(m[:], u0_sb[:], r[:])
            V.scalar_tensor_tensor(s2[:], r[:], A1, s1[:], MULT, ADD)
            V.scalar_tensor_tensor(un[:], m[:], -A1, s2[:], MULT, ADD)

            t2p = wk.tile([P, F], F32, tag="t3", bufs=1)
            t3p = wk.tile([P, F], F32, tag="t4", bufs=1)
            s4 = wk.tile([P, F], F32, tag="t5", bufs=1)
            xn = spool.tile([P, F], F32, tag="x")
            V.tensor_mul(t2p[:], x0_sb[:], r[:])
            V.tensor_mul(t3p[:], un[:], t2p[:])
            V.tensor_scalar(s4[:], x0_sb[:], 1.0 - C2, C2, MULT, ADD)
            V.scalar_tensor_tensor(xn[:], t3p[:], -DT, s4[:], MULT, ADD)

            w0 = wk.tile([P, F], F32, tag="t6", bufs=1)
            V.tensor_mul(w0[:], un[:], xn[:])
            yh = {}
            for hf, sl in (("A", slice(0, HW_)), ("B", slice(HW_, F))):
                yh[hf] = yp.tile([P, HW_], F8, tag=f"y{hf}",
                                 name=f"y{hf}_pro")
                V.scalar_tensor_tensor(yh[hf][:], w0[:, sl], YSCALE,
                                       r[:, sl], MULT, MULT)

            ag_counter = [0]

            def launch_ag(hf, ytile):
                """store y-half to DRAM, AllGather, DMA gathered chunks back."""
                k = ag_counter[0] = ag_counter[0] + 1
                ydr = dp.tile([P, HW_], F8, tag=f"ydr{hf}", name=f"ydr{hf}_{k}")
                nc.scalar.dma_start(ydr[:], ytile[:])
                yall = dp.tile([NCORES, P, HW_], F8, tag=f"yall{hf}",
                               name=f"yall{hf}_{k}", addr_space="Shared")
                nc.gpsimd.collective_compute(
                    "AllGather",
                    mybir.AluOpType.bypass,
                    replica_groups=[list(range(NCORES))],
                    ins=[ydr.opt()],
                    outs=[yall.opt()],
                )
                # 3 chunk tiles -> progressive ungating of the consumers
                y0 = yp.tile([P, HW_], F8, tag=f"yg0{hf}", name=f"yg0{hf}_{k}")
                y1 = yp.tile([P, HW_], F8, tag=f"yg1{hf}", name=f"yg1{hf}_{k}")
                yR = yp.tile([P, 6 * HW_], F8, tag=f"ygR{hf}",
                             name=f"ygR{hf}_{k}")
                nc.sync.dma_start(y0[:], yall[0, :, :])
                nc.sync.dma_start(y1[:], yall[1, :, :])
                nc.sync.dma_start(
                    yR[:].rearrange("p (c f) -> p c f", c=NCORES - 2),
                    yall[2:, :, :].rearrange("c p f -> p c f"),
                )
                return (y0, y1, yR)

            yfA = launch_ag("A", yh["A"])
            yfB = launch_ag("B", yh["B"])

            pdum = pmm.tile([B, 512], F32, tag="dummy", bufs=1,
                            name="pdum") if n_dummy else None
            dum_src = [xn]  # previous step's x' carry: written at ew_B end

            # ---- main loop ----
            for it in range(n_steps):
                last = it == n_steps - 1

                def lhst_ap(t):
                    """y K-tile AP [128, 32]."""
                    c, j = divmod(t, J)
                    yf = yfA if j < 4 else yfB
                    jj = j if j < 4 else j - 4
                    chunk = yf[c] if c < 2 else yf[2]
                    off = (0 if c < 2 else (c - 2) * HW_) + jj * B
                    return chunk[:, off:off + B]

                # precompute (overlaps matmuls on DVE)
                A_t = wk.tile([P, F], F32, tag="A", bufs=1)
                B_t = wk.tile([P, F], F32, tag="B", bufs=1)
                C_t = wk.tile([P, F], F32, tag="C", bufs=1)
                D_t = wk.tile([P, F], F32, tag="D", bufs=1)
                rE = wk.tile([P, F], F32, tag="rE", bufs=1)
                if not last:
                    V.tensor_scalar(A_t[:], un[:], 1.0 - C1, C0, MULT, ADD)
                    V.tensor_scalar(B_t[:], un[:], -A1, A1, MULT, ADD)
                    V.tensor_scalar(C_t[:], xn[:], 1.0 - C2, C2, MULT, ADD)
                    V.tensor_scalar(D_t[:], xn[:], DT, None, MULT)
                if e_v is None:
                    V.tensor_mul(rE[:], r[:], e_sb[:])

                # column-group PSUM tiles: group g accumulates at
                # partitions [32g, 32g+32), each in its own 2 KiB bank
                # (matmul start=True claims a whole zero region).
                pg = [pmm.tile([P, 512], F32, tag=f"pg{g}", bufs=1,
                               name=f"pg{g}_{it}") for g in range(4)]

                def emit_waves(groups, tiles, base_idx):
                    for k_, t in enumerate(tiles):
                        ki = base_idx + k_
                        for g in groups:
                            nc.tensor.matmul(
                                pg[g][32 * g:32 * (g + 1), :GW],
                                lhsT=lhst_ap(t),
                                rhs=w_sb[:, t * NS + g * GW:
                                         t * NS + (g + 1) * GW],
                                start=(ki == 0),
                                stop=(ki == T - 1),
                                tile_position=(0, 32 * g),
                            )

                def transpose_half(hf, groups):
                    """PSUM column-groups -> state-layout PSUM [128, 128]."""
                    mmT_ = pT.tile([P, HW_], F32, tag=f"mmT{hf}", bufs=1,
                                   name=f"mmT{hf}_{it}")
                    stage = wk.tile([P, GW], F32, tag=f"stage{hf}",
                                    bufs=1, name=f"stage{hf}_{it}")
                    for g in groups:
                        nc.scalar.copy(stage[32 * g:32 * (g + 1), :],
                                       pg[g][32 * g:32 * (g + 1), :GW])
                    for jl in range(4):
                        g = groups[jl // 2]
                        jj = jl % 2
                        nc.tensor.transpose(
                            mmT_[:, jl * B:(jl + 1) * B],
                            stage[32 * g:32 * (g + 1),
                                  jj * P:(jj + 1) * P],
                            identF[32 * g:32 * (g + 1), :],
                            tile_position=(32 * g, 0),
                        )
                    return mmT_

                # dummy matmuls fill the AllGather wait at the step end:
                # gated on the previous step's x' carry (written by the
                # last ew_B op), so they cannot run before the gap opens
                # and keep the PE HAM clock at full rate through it.
                if n_dummy and it > 0:
                    ds_t = dum_src[0]
                    for dk in range(n_dummy):
                        nc.tensor.matmul(
                            pdum[:, :F], lhsT=ds_t[:, :B], rhs=ds_t[:],
                            start=True, stop=True,
                        )

                emit_waves((0, 1), KORDER, 0)
                mmTA = transpose_half("A", (0, 1))
                emit_waves((2, 3), KORDER, 0)

                r_new = spool.tile([P, F], F32, tag="r")
                rf_new = spool.tile([P, F], F32, tag="rf")
                q = spool.tile([P, F], F32, tag="u")
                v = spool.tile([P, F], F32, tag="x")
                newy = {"A": yp.tile([P, HW_], F8, tag="yA", name=f"yA_{it}"),
                        "B": yp.tile([P, HW_], F8, tag="yB", name=f"yB_{it}")}

                def ew_half(hf, mmT_half):
                    sl = slice(0, HW_) if hf == "A" else slice(HW_, F)
                    # critical chain: mm -> y
                    h_ = wk.tile([P, HW_], F32, tag=f"w1{hf}", bufs=1)
                    if ds_v is not None:
                        V.scalar_tensor_tensor(h_[:], mmT_half[:],
                                               ds_v / YSCALE,
                                               rf[:, sl], MULT, ADD)
                    else:
                        tmp = wk.tile([P, HW_], F32, tag=f"w0{hf}", bufs=1)
                        V.tensor_mul(tmp[:], mmT_half[:], ds_sb[:, sl])
                        V.scalar_tensor_tensor(h_[:], tmp[:],
                                               1.0 / YSCALE,
                                               rf[:, sl], MULT, ADD)
                    dr_ = wk.tile([P, HW_], F32, tag=f"w2{hf}", bufs=1)
                    if dt_v is not None:
                        V.tensor_scalar(dr_[:], h_[:], 0.0, dt_v, MAX, MULT)
                    else:
                        V.scalar_tensor_tensor(dr_[:], h_[:], 0.0, dt_sb[:, sl],
                                               MAX, MULT)
                    if e_v is not None:
                        V.scalar_tensor_tensor(r_new[:, sl], r[:, sl], e_v,
                                               dr_[:], MULT, ADD)
                    else:
                        V.tensor_add(r_new[:, sl], dr_[:], rE[:, sl])
                    if last:
                        return None
                    m1_ = wk.tile([P, HW_], F32, tag=f"w3{hf}", bufs=1)
                    V.tensor_mul(m1_[:], B_t[:, sl], r_new[:, sl])
                    V.tensor_add(q[:, sl], m1_[:], A_t[:, sl])
                    tt_ = wk.tile([P, HW_], F32, tag=f"w4{hf}", bufs=1)
                    V.tensor_mul(tt_[:], r_new[:, sl], q[:, sl])
                    s2_ = wk.tile([P, HW_], F32, tag=f"w5{hf}", bufs=1)
                    V.tensor_mul(s2_[:], D_t[:, sl], tt_[:])
                    V.scalar_tensor_tensor(v[:, sl], s2_[:], -1.0, C_t[:, sl],
                                           MULT, ADD)
                    ynew = newy[hf]
                    V.scalar_tensor_tensor(ynew[:], tt_[:], YSCALE,
                                           v[:, sl], MULT, MULT)
                    # off critical path: rf' = es*h + fme
                    if es_v is not None:
                        V.scalar_tensor_tensor(rf_new[:, sl], h_[:], es_v,
                                               fme[:, sl], MULT, ADD)
                    else:
                        tmp2 = wk.tile([P, HW_], F32, tag=f"w6{hf}", bufs=1)
                        V.tensor_mul(tmp2[:], h_[:], es_sb[:, sl])
                        V.tensor_add(rf_new[:, sl], tmp2[:], fme[:, sl])
                    return ynew

                yA_next = ew_half("A", mmTA)
                if not last:
                    nextA = launch_ag("A", yA_next)

                mmTB = transpose_half("B", (2, 3))
                yB_next = ew_half("B", mmTB)
                if not last:
                    nextB = launch_ag("B", yB_next)
                    yfA, yfB = nextA, nextB
                    un, xn, rf = q, v, rf_new
                    dum_src[0] = v
                    yh = newy
                r = r_new

            # ---- epilogue ----
            for qi in range(4):
                nc.sync.dma_start(
                    r_out[32 * qi:32 * (qi + 1), :],
                    r[32 * qi:32 * (qi + 1), :],
                )

    nc.compile()
    return nc


# ---------------------------------------------------------------------------
# host-side data marshalling
# ---------------------------------------------------------------------------

def _shard_state(v, c):
    """[B, N] float array -> core c state tile [128, 256] (f32)."""
    vs = np.asarray(v, np.float32)[:, c * NS:(c + 1) * NS]      # [32, 1024]
    return np.ascontiguousarray(
        vs.reshape(B, J, P).transpose(2, 1, 0).reshape(P, F)
    )


def _shard_vec(v, c):
    """[N] float vector -> replicated core c tile [128, 256] (f32)."""
    vs = np.asarray(v, np.float32)[c * NS:(c + 1) * NS].reshape(J, P)  # [j, p]
    t = vs.T[:, :, None]                                        # [p, j, 1]
    return np.ascontiguousarray(np.broadcast_to(t, (P, J, B)).reshape(P, F))


def _shard_w(Wab, c):
    """Wab [N, N] -> core c weight tiles [64, 128, 1024] fp16.

    w[t, p, n] = Wab[c*1024 + n, t*128 + p]
    """
    wt = np.asarray(Wab, np.float32)[c * NS:(c + 1) * NS, :].T  # [8192, 1024]
    return np.ascontiguousarray(wt.astype(np.float16).reshape(T, P, NS))


def _unshard_out(tiles):
    """list of 8 [128, 256] tiles -> [32, 8192] f32."""
    out = np.empty((B, N), np.float32)
    for c, tl in enumerate(tiles):
        out[:, c * NS:(c + 1) * NS] = (
            np.asarray(tl, np.float32).reshape(P, J, B).transpose(2, 1, 0)
            .reshape(B, NS)
        )
    return out


def make_in_maps(rates, rec_input, ff_input, Wab, u_stp, x_stp,
                 exp_dt_tau, dt_tau, exp_dt_tau_syn, dt_tau_syn):
    recs_full = (np.asarray(exp_dt_tau_syn, np.float32)[None, :]
                 * np.asarray(rec_input, np.float32))
    in_maps = []
    for c in range(NCORES):
        in_maps.append({
            "w": _shard_w(Wab, c),
            "r0": _shard_state(rates, c),
            "recs0": _shard_state(recs_full, c),
            "u0": _shard_state(u_stp, c),
            "x0": _shard_state(x_stp, c),
            "ff": _shard_state(ff_input, c),
            "es": _shard_vec(exp_dt_tau_syn, c),
            "ds": _shard_vec(dt_tau_syn, c),
            "e": _shard_vec(exp_dt_tau, c),
            "dt": _shard_vec(dt_tau, c),
        })
    return in_maps


_PROGRAM_CACHE = {}


def _uniform_val(v):
    v = np.asarray(v, np.float32)
    return float(v.flat[0]) if np.all(v == v.flat[0]) else None


def _get_program(n_steps, uni):
    key = (n_steps, uni)
    if key not in _PROGRAM_CACHE:
        _PROGRAM_CACHE[key] = build_program(n_steps, uni=uni)
    return _PROGRAM_CACHE[key]


def run(trace=False, tmpdir=None, **inputs):
    n_steps = int(inputs.pop("n_steps"))
    uni = (_uniform_val(inputs["exp_dt_tau_syn"]),
           _uniform_val(inputs["dt_tau_syn"]),
           _uniform_val(inputs["exp_dt_tau"]),
           _uniform_val(inputs["dt_tau"]))
    nc = _get_program(n_steps, uni)
    in_maps = make_in_maps(**inputs)
    res = bass_utils.run_bass_kernel_spmd(
        nc, in_maps, core_ids=list(range(NCORES)), trace=trace, tmpdir=tmpdir
    )
    out = _unshard_out([m["r_out"] for m in res.results])
    return out, res


def kernel(**inputs):
    out, _ = run(**inputs)
    return out
